# revision 5
# baseline (speedup 1.0000x reference)
"""ConstellationNet VQ-codebook clustering kernel for 8x Trainium2 NeuronCores.

Reference semantics (per iteration):
    d2 = x2 - 2 x@v.T + v2;  d = sqrt(max(d2, 0))
    m = softmax(-beta*d, axis=1)          (beta=100 -> essentially one-hot argmin)
    msum = m.sum(0)                        (all-reduced across cores)
    v_p = msum * x_head_sum / msum         (NaN when msum==0 -> poisons v)
    mu = 1/(s+msum);  v = (1-mu) v + mu v_p;  s += msum
Output: d from the LAST iteration (computed from v after epoch-1 updates).

Sharding: N = B*H*W = 102400 points split contiguously across 8 cores
(12800 each); (k,C) centroid table replicated; per-iteration [K] soft-count
sums all-reduced (512B AllReduce).

Device numerics: iterations 0..epoch-2 classify points by argmin distance
(beta=100 makes softmax one-hot to ~1e-5) using bf16 matmuls; the final
iteration computes d with fp32 matmuls. A NaN "poison" scalar derived from
v's norms is folded into msum each iteration so that once any centroid goes
NaN (msum==0 -> 0*inf in v_p, as in the reference), all centroids go NaN on
the next step, exactly like the reference's row-softmax NaN propagation.
NaNs in the final d are flushed to 0 like jnp.maximum(d2,0)+sqrt lowers on
the neuron backend.
"""
import sys
import numpy as np

sys.path.insert(0, "/opt/trn_rl_repo")

B, H, C, K = 64, 40, 128, 128
N = B * H * H            # 102400
NCORES = 8
NSHARD = N // NCORES     # 12800
NTILES = NSHARD // 128   # 100
NCHUNK = 10              # xT split into chunks of [128, 1280]
TPC = NTILES // NCHUNK   # tiles per chunk = 10
GROUPS = NTILES // 4     # 25 groups of 4 tiles
BETA, LBDA = 100.0, 1.0

_CACHE = {}


def _build(epoch: int):
    import concourse.bass as bass
    import concourse.bacc as bacc
    import concourse.tile as tile
    import concourse.mybir as mybir

    F32 = mybir.dt.float32
    BF16 = mybir.dt.bfloat16
    AF = mybir.ActivationFunctionType
    ALU = mybir.AluOpType
    AX = mybir.AxisListType
    ts = bass.ts

    nc = bacc.Bacc("TRN2", target_bir_lowering=False, debug=False,
                   num_devices=NCORES)

    xT_in = nc.dram_tensor("xT", [C, NSHARD], F32, kind="ExternalInput")
    xheadT_in = nc.dram_tensor("xheadT", [C, K], F32, kind="ExternalInput")
    vT0_in = nc.dram_tensor("vT0", [C, K], F32, kind="ExternalInput")
    d_out = nc.dram_tensor("d", [NSHARD, K], F32, kind="ExternalOutput")
    flag_out = nc.dram_tensor("flag", [1, 1], F32, kind="ExternalOutput")

    # DRAM output viewed as [group, partition(=row within 128-tile), j, k]
    d_out_r = d_out.ap().rearrange("(G j p) k -> G p j k", j=4, p=128)

    with tile.TileContext(nc) as tc:
        from contextlib import ExitStack
        es = ExitStack()
        per = es.enter_context(tc.tile_pool(name="persist", bufs=1))

        def ptile(name, shape, dtype):
            return per.tile(shape, dtype, name=name, tag=name)

        # ---- persistent SBUF state ----
        xf = [ptile(f"xf{c}", [C, NSHARD // NCHUNK], F32) for c in range(NCHUNK)]
        xb = [ptile(f"xb{c}", [C, NSHARD // NCHUNK], BF16) for c in range(NCHUNK)]
        vT = ptile("vT", [C, K], F32)
        xhs_row = ptile("xhs_row", [1, K], F32)
        s_row = ptile("s_row", [1, K], F32)
        x2q = ptile("x2q", [128, NTILES], F32)
        ones_col_f = ptile("ones_col_f", [128, 1], F32)
        ones_col_b = ptile("ones_col_b", [128, 1], BF16)
        ones_row_f = ptile("ones_row_f", [1, 128], F32)
        ones_row_b = ptile("ones_row_b", [1, 128], BF16)
        xheadT = ptile("xheadT_sb", [C, K], F32)

        nc.vector.memset(ones_col_f[:], 1.0)
        nc.vector.memset(ones_col_b[:], 1.0)
        nc.vector.memset(ones_row_f[:], 1.0)
        nc.vector.memset(ones_row_b[:], 1.0)
        nc.vector.memset(s_row[:], 0.0)

        # ---- pools ----
        psd2 = es.enter_context(tc.tile_pool(name="psd2", bufs=3, space="PSUM"))
        psms = es.enter_context(tc.tile_pool(name="psms", bufs=1, space="PSUM"))
        psv2 = es.enter_context(tc.tile_pool(name="psv2", bufs=2, space="PSUM"))
        psx2 = es.enter_context(tc.tile_pool(name="psx2", bufs=1, space="PSUM"))
        psmv = es.enter_context(tc.tile_pool(name="psmv", bufs=1, space="PSUM"))
        work = es.enter_context(tc.tile_pool(name="work", bufs=3))
        rows = es.enter_context(tc.tile_pool(name="rows", bufs=2))
        dram = es.enter_context(tc.tile_pool(name="dram", bufs=2, space="DRAM"))

        # ---- load + cast input ----
        nc.sync.dma_start(xheadT[:], xheadT_in.ap())
        nc.sync.dma_start(vT[:], vT0_in.ap())
        CW = NSHARD // NCHUNK
        for c in range(NCHUNK):
            nc.sync.dma_start(xf[c][:], xT_in.ap()[:, ts(c, CW)])
            nc.vector.tensor_copy(xb[c][:], xf[c][:])

        # xhs_row = column sums of xheadT (= row sums of x_head)
        ph = psv2.tile([1, K], F32, name="ph", tag="prow")
        nc.tensor.matmul(ph[:], ones_col_f[:], xheadT[:], start=True, stop=True)
        nc.scalar.copy(xhs_row[:], ph[:])

        def xslice_f(i):
            return xf[i // TPC][:, ts(i % TPC, 128)]

        def xslice_b(i):
            return xb[i // TPC][:, ts(i % TPC, 128)]

        # ---------------- iterations 0 .. epoch-2 (bf16, count-based msum) ----
        n_sign_tiles = 0
        for t in range(epoch - 1):
            # v-side prep
            vsq = work.tile([C, K], F32, name="vsq", tag="vsq")
            nc.scalar.activation(vsq[:], vT[:], AF.Square)
            pv2 = psv2.tile([1, K], F32, name="pv2", tag="prow")
            nc.tensor.matmul(pv2[:], ones_col_f[:], vsq[:], start=True, stop=True)
            v2row_b = rows.tile([1, K], BF16, name="v2row_b", tag="v2row_b")
            nc.scalar.copy(v2row_b[:], pv2[:])
            # poison scalar: 0 * sum(v2) -> NaN iff v has any NaN, else 0.0
            pzs = rows.tile([1, 1], F32, name="pzs", tag="pzs")
            nc.vector.tensor_reduce(pzs[:], pv2[:], axis=AX.X, op=ALU.add)
            pz0 = rows.tile([1, 1], F32, name="pz0", tag="pz0")
            nc.vector.tensor_scalar_mul(pz0[:], pzs[:], 0.0)
            vm2b = work.tile([C, K], BF16, name="vm2b", tag="vm2b")
            nc.vector.tensor_scalar_mul(vm2b[:], vT[:], -2.0)

            pms = psms.tile([1, 4 * 128], F32, name="pms", tag="pms")
            for g in range(GROUPS):
                pd = psd2.tile([128, 4, 128], F32, name="pd", tag="pd")
                for j in range(4):
                    nc.tensor.matmul(pd[:, j, :], xslice_b(4 * g + j), vm2b[:],
                                     start=True, stop=False)
                    nc.tensor.matmul(pd[:, j, :], ones_row_b[:], v2row_b[:],
                                     start=False, stop=True)
                d2min = work.tile([128, 4], F32, name="d2min", tag="d2min")
                nc.vector.tensor_reduce(d2min[:], pd[:], axis=AX.X, op=ALU.min)
                z = work.tile([128, 4, 128], BF16, name="z", tag="z")
                for j in range(3):
                    # sign(d2min - d2): 0 at the argmin, -1 elsewhere
                    nc.scalar.activation(z[:, j, :], pd[:, j, :], AF.Sign,
                                         bias=d2min[:, j:j + 1], scale=-1.0)
                    n_sign_tiles += 1
                # is_equal(d2, d2min): 1 at the argmin, 0 elsewhere
                nc.vector.tensor_scalar(z[:, 3, :], pd[:, 3, :],
                                        d2min[:, 3:4], None, op0=ALU.is_equal)
                nc.tensor.matmul(pms[:], ones_col_b[:],
                                 z[:].rearrange("p j k -> p (j k)"),
                                 start=(g == 0), stop=(g == GROUPS - 1))

            # fold [1, (j k)] -> [1, k] summing over j, then + counts offset
            msum_loc = rows.tile([1, K], F32, name="msum_loc", tag="msum_loc")
            nc.vector.tensor_reduce(
                msum_loc[:], pms[:].rearrange("p (j k) -> p k j", j=4),
                axis=AX.X, op=ALU.add)
            nc.vector.tensor_scalar_add(msum_loc[:], msum_loc[:],
                                        float(3 * GROUPS * 128))

            # all-reduce the per-core counts
            ar_in = dram.tile([1, K], F32, name="ar_in", tag="ar_in")
            ar_out = dram.tile([1, K], F32, name="ar_out", tag="ar_out")
            nc.gpsimd.dma_start(ar_in[:], msum_loc[:])
            nc.gpsimd.collective_compute(
                "AllReduce", ALU.add,
                replica_groups=[list(range(NCORES))],
                ins=[ar_in.opt()], outs=[ar_out.opt()])
            msum = rows.tile([1, K], F32, name="msum", tag="msum")
            nc.gpsimd.dma_start(msum[:], ar_out[:])

            # poison injection (NaN spread like reference row-softmax)
            nc.vector.tensor_scalar(msum[:], msum[:], pz0[:1, :1], None,
                                    op0=ALU.add)

            # v update (row space, replicated on all cores)
            nc.vector.tensor_add(s_row[:], s_row[:], msum[:])
            mu_row = rows.tile([1, K], F32, name="mu_row", tag="mu_row")
            nc.vector.reciprocal(mu_row[:], s_row[:])
            rminv = rows.tile([1, K], F32, name="rminv", tag="rminv")
            nc.vector.reciprocal(rminv[:], msum[:])
            vp_row = rows.tile([1, K], F32, name="vp_row", tag="vp_row")
            nc.vector.tensor_mul(vp_row[:], msum[:], xhs_row[:])
            nc.vector.tensor_mul(vp_row[:], vp_row[:], rminv[:])
            pmv = psmv.tile([128, 2, K], F32, name="pmv", tag="pmv")
            nc.tensor.matmul(pmv[:, 0, :], ones_row_f[:], mu_row[:],
                             start=True, stop=True)
            nc.tensor.matmul(pmv[:, 1, :], ones_row_f[:], vp_row[:],
                             start=True, stop=True)
            dv = work.tile([C, K], F32, name="dv", tag="dv")
            nc.vector.tensor_sub(dv[:], pmv[:, 1, :], vT[:])
            nc.vector.tensor_mul(dv[:], pmv[:, 0, :], dv[:])
            nc.vector.tensor_add(vT[:], vT[:], dv[:])

            if t == 0:
                # x2 per point (needed only for the final iteration's sqrt
                # bias); emitted here so it fills engine slack early.
                px2 = psx2.tile([128, NTILES], F32, name="px2", tag="px2")
                for c in range(NCHUNK):
                    sq = work.tile([C, CW], F32, name="sq", tag="sq")
                    nc.scalar.activation(sq[:], xf[c][:], AF.Square)
                    for i in range(TPC):
                        nc.tensor.matmul(px2[:, c * TPC + i:c * TPC + i + 1],
                                         sq[:, ts(i, 128)], ones_col_f[:],
                                         start=True, stop=True)
                nc.vector.tensor_copy(x2q[:], px2[:])

        # ---------------- final iteration: d = sqrt(d2), fp32 ----------------
        vm2f = work.tile([C, K], F32, name="vm2f", tag="vm2f")
        nc.vector.tensor_scalar_mul(vm2f[:], vT[:], -2.0)
        vsqf = work.tile([C, K], F32, name="vsqf", tag="vsqf")
        nc.scalar.activation(vsqf[:], vT[:], AF.Square)
        pv2f = psv2.tile([1, K], F32, name="pv2f", tag="prow")
        nc.tensor.matmul(pv2f[:], ones_col_f[:], vsqf[:], start=True, stop=True)
        v2row_f = rows.tile([1, K], F32, name="v2row_f", tag="v2row_f")
        nc.scalar.copy(v2row_f[:], pv2f[:])
        # final poison flag (NaN iff final v has any NaN)
        pzf = rows.tile([1, 1], F32, name="pzf", tag="pzf")
        nc.vector.tensor_reduce(pzf[:], pv2f[:], axis=AX.X, op=ALU.add)
        nc.vector.tensor_scalar_mul(pzf[:], pzf[:], 0.0)
        nc.sync.dma_start(flag_out.ap(), pzf[:])

        for g in range(GROUPS):
            pd = psd2.tile([128, 4, 128], F32, name="pdf", tag="pd")
            for j in range(4):
                nc.tensor.matmul(pd[:, j, :], xslice_f(4 * g + j), vm2f[:],
                                 start=True, stop=False)
                nc.tensor.matmul(pd[:, j, :], ones_row_f[:], v2row_f[:],
                                 start=False, stop=True)
            dt = work.tile([128, 4, 128], F32, name="dt", tag="dt")
            for j in range(4):
                i = 4 * g + j
                nc.scalar.activation(dt[:, j, :], pd[:, j, :], AF.Sqrt,
                                     bias=x2q[:, i:i + 1], scale=1.0)
            nc.sync.dma_start(d_out_r[g], dt[:])

        es.close()

    nc.compile()
    return nc


def kernel(cellFeature, v_init, epoch):
    from concourse.bass_utils import run_bass_kernel_spmd

    epoch = int(np.asarray(epoch))
    x = np.ascontiguousarray(np.asarray(cellFeature, dtype=np.float32)).reshape(N, C)
    v0 = np.asarray(v_init, dtype=np.float32)
    if epoch <= 0:
        return np.zeros((B, H, H, K), np.float32)

    if epoch not in _CACHE:
        _CACHE[epoch] = _build(epoch)
    nc = _CACHE[epoch]

    xheadT = np.ascontiguousarray(x[:K].T)
    vT0 = np.ascontiguousarray(v0.T)
    in_maps = []
    for c in range(NCORES):
        in_maps.append({
            "xT": np.ascontiguousarray(x[c * NSHARD:(c + 1) * NSHARD].T),
            "xheadT": xheadT,
            "vT0": vT0,
        })

    res = run_bass_kernel_spmd(nc, in_maps, core_ids=list(range(NCORES)))
    d = np.concatenate([r["d"] for r in res.results], axis=0)
    # NaN -> 0, matching sqrt(maximum(d2, 0)) lowering on the neuron backend
    # (reference semantics once a centroid's soft-count underflows to 0).
    d = np.where(np.isnan(d), np.float32(0.0), d)
    return d.reshape(B, H, H, K).astype(np.float32)


def _in_maps(cellFeature, v_init):
    x = np.ascontiguousarray(np.asarray(cellFeature, dtype=np.float32)).reshape(N, C)
    v0 = np.asarray(v_init, dtype=np.float32)
    xheadT = np.ascontiguousarray(x[:K].T)
    vT0 = np.ascontiguousarray(v0.T)
    return [{
        "xT": np.ascontiguousarray(x[c * NSHARD:(c + 1) * NSHARD].T),
        "xheadT": xheadT,
        "vT0": vT0,
    } for c in range(NCORES)]


def run_traced(inputs):
    """Dev helper: run with NTFF tracing to get HW exec time (test.py only)."""
    from concourse.bass_utils import run_bass_kernel_spmd
    epoch = int(np.asarray(inputs["epoch"]))
    if epoch not in _CACHE:
        _CACHE[epoch] = _build(epoch)
    return run_bass_kernel_spmd(
        _CACHE[epoch], _in_maps(inputs["cellFeature"], inputs["v_init"]),
        core_ids=list(range(NCORES)), trace=True)
